# Initial kernel scaffold
#
"""BiUTE kernel for Trainium2, 8-core data-parallel over batch.

Math (per batch element b, T=128, N=12, D=1024, F=2D=2048):
  u = Wq.sum(0)                                  [D]
  w[t,n]  = sum_d feat[t,n,d] * u[d]             [T,N]
  g[t,d]  = sum_n w[t,n] * feat[t,n,d]           [T,D]
  f[t,d]  = max_n feat[t,n,d]                    [T,D]
  n = [g | f]                                    [T,F]
  tb = n @ Wtb.T ; pb = n @ Wpb.T ; gb = n @ Wgb.T
  sb = (tb @ pb.T) * scale ; out_b = (sb*lower) @ gb
  (same for 'after' branch with upper mask)
  out = n + out_b + out_a                        [T,F]

Sharding: B=16 split 2 per core across 8 cores; weights replicated.
fp16 matmul operands, fp32 accumulation/output. w runs on the tensor
engine against DMA-transposed features; weights stream through three
2.1MB SBUF slots on the scalar HWDGE ring; the f-half of n is produced
first so projection matmuls start before g finishes on the vector
engine.
"""

import numpy as np

import concourse.mybir as mybir
import concourse.tile as tile
from concourse import bacc
from concourse.bass_utils import run_bass_kernel_spmd

F32 = mybir.dt.float32
F16 = mybir.dt.float16

B, T, NP, D = 16, 128, 12, 1024
F = 2 * D                      # 2048
FC_ORDER = list(range(8, 16)) + list(range(8))  # f-half of n first
NB = 2                         # batch elements per core
NCORES = 8
TN = T * NP                    # 1536 flattened (t, n) rows
NFC = F // 128                 # 16 f-chunks of nT
SCALE = 1.0 / float(np.sqrt(F))

_CACHE = {}


def _build():
    nc = bacc.Bacc("TRN2", target_bir_lowering=False, debug=False)
    mult = mybir.AluOpType.mult
    add = mybir.AluOpType.add

    featd = nc.dram_tensor("feat", [NB, TN, D], F16, kind="ExternalInput")
    ud = nc.dram_tensor("u", [1, D], F16, kind="ExternalInput")
    mbd = nc.dram_tensor("maskb", [T, T], F32, kind="ExternalInput")
    mad = nc.dram_tensor("maska", [T, T], F32, kind="ExternalInput")
    identd = nc.dram_tensor("ident", [128, 128], F32, kind="ExternalInput")
    wtpbd = nc.dram_tensor("wtp_b", [F, F], F16, kind="ExternalInput")
    wgbd = nc.dram_tensor("wg_b", [F, F], F16, kind="ExternalInput")
    wtpad = nc.dram_tensor("wtp_a", [F, F], F16, kind="ExternalInput")
    wgad = nc.dram_tensor("wg_a", [F, F], F16, kind="ExternalInput")
    outd = nc.dram_tensor("out", [NB, T, F], F32, kind="ExternalOutput")

    with tile.TileContext(nc) as tc:
        with (
            tc.tile_pool(name="consts", bufs=1) as consts,
            tc.tile_pool(name="npool", bufs=1) as npool,
            tc.tile_pool(name="ntpool", bufs=1) as ntpool,
            tc.tile_pool(name="wres", bufs=4) as wsp,
            tc.tile_pool(name="drains", bufs=1) as drp,
            tc.tile_pool(name="sbp", bufs=2) as sbp,
            tc.tile_pool(name="featp", bufs=2) as featp,
            tc.tile_pool(name="aw", bufs=2) as awp,
            tc.tile_pool(name="ps2", bufs=2, space="PSUM") as ps2p,
        ):
            u_sb = consts.tile([128, D], F16)
            nc.gpsimd.dma_start(out=u_sb[:], in_=ud[:].to_broadcast((128, D)))
            mb_sb = consts.tile([T, T], F32)
            nc.sync.dma_start(out=mb_sb[:], in_=mbd[:])
            ma_sb = consts.tile([T, T], F32)
            nc.sync.dma_start(out=ma_sb[:], in_=mad[:])
            ident = consts.tile([128, 128], F32)
            nc.sync.dma_start(out=ident[:], in_=identd[:])

            n_sb = [
                npool.tile([T, F], F32, tag=f"n{b}", name=f"n{b}")
                for b in range(NB)
            ]
            nT = ntpool.tile([128, NFC, NB * T], F16)

            # Feature loads: t-major copies on the sync ring.
            feats = []
            for b in range(NB):
                feat = featp.tile([T, NP, D], F16, tag="feat", name=f"feat{b}")
                nc.sync.dma_start(
                    out=feat[:, :, : D // 2],
                    in_=featd[b, :, : D // 2].rearrange("(p c) d -> p c d", c=NP),
                )
                nc.scalar.dma_start(
                    out=feat[:, :, D // 2 :],
                    in_=featd[b, :, D // 2 :].rearrange("(p c) d -> p c d", c=NP),
                )
                feats.append(feat)

            def load_w_quarter(src, qc, name):
                """Stream one [2048, 512] fp16 weight column-quarter."""
                wh = wsp.tile([128, NFC, 512], F16, tag="w", name=name)
                for q in range(3, -1, -1):  # later fc chunks first
                    nc.scalar.dma_start(
                        out=wh[:, 4 * q : 4 * (q + 1), :],
                        in_=src[
                            512 * q : 512 * (q + 1), 512 * qc : 512 * (qc + 1)
                        ].rearrange("(c p) e -> p c e", p=128),
                    )
                return wh

            def emit_pass2(wgh, qc, b, gb16):
                """gb[:, qc-quarter] = n_b @ Wg[qc].T  (natural layout)."""
                psg = ps2p.tile(
                    [128, 512], F32, tag=f"psg{b}", name=f"psg{b}_{qc}"
                )
                for i, fc in enumerate(FC_ORDER):
                    nc.tensor.matmul(
                        psg[:],
                        nT[:, fc, T * b : T * (b + 1)],
                        wgh[:, fc, :],
                        start=(i == 0),
                        stop=(i == NFC - 1),
                    )
                nc.scalar.copy(
                    gb16[b][:, 512 * qc : 512 * (qc + 1)], psg[:]
                )

            def emit_transp(b, fc):
                pt = psAt.tile([128, 128], F32, tag="pt", name="pt")
                nc.tensor.transpose(
                    pt[:], n_sb[b][:, 128 * fc : 128 * (fc + 1)], ident[:]
                )
                nc.scalar.copy(nT[:, fc, T * b : T * (b + 1)], pt[:])

            def emit_f(b):
                feat = feats[b]
                facc = awp.tile([T, D], F16, tag="facc", name=f"facc{b}")
                nc.vector.tensor_max(facc[:], feat[:, 0, :], feat[:, 1, :])
                for c in range(2, NP):
                    nc.vector.tensor_max(facc[:], facc[:], feat[:, c, :])
                nc.scalar.copy(n_sb[b][:, D:], facc[:])
                for fc in range(8, NFC):
                    emit_transp(b, fc)

            def emit_w(b):
                feat = feats[b]
                wvec = awp.tile([T, NP], F32, tag=f"wvec{b}", name=f"wvec{b}")
                scr = awp.tile([T, D], F16, tag="scr", name=f"scr{b}")
                for c in range(NP):
                    nc.vector.scalar_tensor_tensor(
                        out=scr[:],
                        in0=feat[:, c, :],
                        scalar=1.0,
                        in1=u_sb[:],
                        op0=mult,
                        op1=mult,
                        accum_out=wvec[:, c : c + 1],
                    )
                return wvec

            def emit_g(b, wvec):
                feat = feats[b]
                nc.vector.tensor_scalar_mul(
                    n_sb[b][:, :D], feat[:, 0, :], wvec[:, 0:1]
                )
                for c in range(1, NP):
                    nc.vector.scalar_tensor_tensor(
                        out=n_sb[b][:, :D],
                        in0=feat[:, c, :],
                        scalar=wvec[:, c : c + 1],
                        in1=n_sb[b][:, :D],
                        op0=mult,
                        op1=add,
                    )
                for fc in range(8):
                    emit_transp(b, fc)

            # ---------------- Phase A + early pass2 ----------------
            gb16_b = [
                drp.tile([T, F], F16, tag=f"gb{b}", name=f"gb{b}_0")
                for b in range(NB)
            ]
            wg_b_q0 = load_w_quarter(wgbd, 0, "wg_b_q0")
            with (
                tc.tile_pool(name="psAt", bufs=2, space="PSUM") as psAt,
            ):
                emit_f(0)
                wv0 = emit_w(0)
                emit_g(0, wv0)
                emit_pass2(wg_b_q0, 0, 0, gb16_b)
                emit_f(1)
                wv1 = emit_w(1)
                emit_g(1, wv1)
                emit_pass2(wg_b_q0, 0, 1, gb16_b)
                for qc in range(1, 4):
                    wgh = load_w_quarter(wgbd, qc, f"wg_b_q{qc}")
                    for b in range(NB):
                        emit_pass2(wgh, qc, b, gb16_b)

            # ---------------- Branches ----------------
            def emit_pass1(wtpd, tp2, sfx):
                for qc in range(4):  # e-col quarters: 0,1 tb; 2,3 pb
                    wth = load_w_quarter(wtpd, qc, f"wt{sfx}_{qc}")
                    with tc.tile_pool(
                        name="ps1", bufs=2, space="PSUM"
                    ) as ps1p:
                        for e4 in range(4):
                            p1 = ps1p.tile([128, 2 * T], F32, tag="p1",
                                           name="p1")
                            for i, fc in enumerate(FC_ORDER):
                                nc.tensor.matmul(
                                    p1[:],
                                    wth[:, fc, 128 * e4 : 128 * (e4 + 1)],
                                    nT[:, fc, :],
                                    start=(i == 0),
                                    stop=(i == NFC - 1),
                                )
                            nc.scalar.copy(tp2[:, 4 * qc + e4, :], p1[:])

            def emit_pass3(tp2, gb16, mask_sb):
                with (
                    tc.tile_pool(name="ps3", bufs=2, space="PSUM") as ps3p,
                    tc.tile_pool(name="ps4", bufs=2, space="PSUM") as ps4p,
                ):
                    for b in range(NB):
                        psb = ps3p.tile([T, T], F32, tag="psb", name="psb")
                        for ec in range(8):
                            nc.tensor.matmul(
                                psb[:],
                                tp2[:, 8 + ec, T * b : T * (b + 1)],
                                tp2[:, ec, T * b : T * (b + 1)],
                                start=(ec == 0),
                                stop=(ec == 7),
                            )
                        sbm = sbp.tile([T, T], F16, tag="sbm", name="sbm")
                        nc.vector.scalar_tensor_tensor(
                            out=sbm[:],
                            in0=psb[:],
                            scalar=1.0,
                            in1=mask_sb[:],
                            op0=mult,
                            op1=mult,
                        )
                        for h4 in range(4):
                            po = ps4p.tile([T, 512], F32, tag="po", name="po")
                            nc.tensor.matmul(
                                po[:],
                                sbm[:],
                                gb16[b][:, 512 * h4 : 512 * (h4 + 1)],
                                start=True,
                                stop=True,
                            )
                            nc.vector.tensor_add(
                                n_sb[b][:, 512 * h4 : 512 * (h4 + 1)],
                                n_sb[b][:, 512 * h4 : 512 * (h4 + 1)],
                                po[:],
                            )

            tp2_b = drp.tile([128, 16, 2 * T], F16, tag="tp2", name="tp2_0")
            emit_pass1(wtpbd, tp2_b, "_0")
            emit_pass3(tp2_b, gb16_b, mb_sb)

            gb16_a = [
                drp.tile([T, F], F16, tag=f"gb{b}", name=f"gb{b}_1")
                for b in range(NB)
            ]
            for qc in range(4):
                wg_a = load_w_quarter(wgad, qc, f"wg_a_q{qc}")
                for b in range(NB):
                    emit_pass2(wg_a, qc, b, gb16_a)
            tp2_a = drp.tile([128, 16, 2 * T], F16, tag="tp2", name="tp2_1")
            emit_pass1(wtpad, tp2_a, "_1")
            emit_pass3(tp2_a, gb16_a, ma_sb)

            for b in range(NB):
                nc.sync.dma_start(out=outd[b], in_=n_sb[b][:])

    nc.compile()
    return nc


def _host_prep(features, Wq, Wtb, Wpb, Wgb, Wta, Wpa, Wga):
    f32 = np.float32
    f16 = np.float16
    feat = np.ascontiguousarray(np.asarray(features, f32)).reshape(B, TN, D)
    u = np.asarray(Wq, f32).sum(axis=0)[None, :]

    def wt(w):  # [e, f] -> [f, e] fp16 contiguous
        return np.ascontiguousarray(np.asarray(w, f32).T.astype(f16))

    wtp_b = np.concatenate([wt(Wtb), wt(Wpb)], axis=1)
    wtp_a = np.concatenate([wt(Wta), wt(Wpa)], axis=1)
    wg_b = wt(Wgb)
    wg_a = wt(Wga)

    idx = np.arange(T)
    maskb = (SCALE * (idx[None, :] > idx[:, None])).astype(f32)  # [j, i]
    maska = (SCALE * (idx[None, :] < idx[:, None])).astype(f32)
    ident = np.eye(128, dtype=f32)

    shared = {
        "u": u.astype(f16),
        "maskb": maskb,
        "maska": maska,
        "ident": ident,
        "wtp_b": wtp_b,
        "wg_b": wg_b,
        "wtp_a": wtp_a,
        "wg_a": wg_a,
    }
    feat16 = feat.astype(f16).reshape(NCORES, NB, TN, D)
    return shared, feat16


def kernel(**inputs) -> np.ndarray:
    if "nc" not in _CACHE:
        _CACHE["nc"] = _build()
    nc = _CACHE["nc"]

    shared, feat16 = _host_prep(**inputs)
    in_maps = [dict(shared, feat=feat16[c]) for c in range(NCORES)]
    res = run_bass_kernel_spmd(nc, in_maps, core_ids=list(range(NCORES)))
    out = np.stack([res.results[c]["out"] for c in range(NCORES)], axis=0)
    return out.reshape(B, T, F).astype(np.float32)



# revision 3
# speedup vs baseline: 2.2642x; 2.2642x over previous
"""BiUTE kernel for Trainium2, 8-core data-parallel over batch.

Math (per batch element b, T=128, N=12, D=1024, F=2D=2048):
  u = Wq.sum(0)                                  [D]
  w[t,n]  = sum_d feat[t,n,d] * u[d]             [T,N]
  g[t,d]  = sum_n w[t,n] * feat[t,n,d]           [T,D]
  f[t,d]  = max_n feat[t,n,d]                    [T,D]
  n = [g | f]                                    [T,F]
  tb = n @ Wtb.T ; pb = n @ Wpb.T ; gb = n @ Wgb.T
  sb = (tb @ pb.T) * scale ; out_b = (sb*lower) @ gb
  (same for 'after' branch with upper mask)
  out = n + out_b + out_a                        [T,F]

Sharding: B=16 split 2 per core across 8 cores; weights replicated.
fp16 matmul operands, fp32 accumulation/output. w runs on the tensor
engine against DMA-transposed features; weights stream through three
2.1MB SBUF slots on the scalar HWDGE ring; the f-half of n is produced
first so projection matmuls start before g finishes on the vector
engine.
"""

import numpy as np

import concourse.mybir as mybir
import concourse.tile as tile
from concourse import bacc
from concourse.bass_utils import run_bass_kernel_spmd

F32 = mybir.dt.float32
F16 = mybir.dt.float16

B, T, NP, D = 16, 128, 12, 1024
F = 2 * D                      # 2048
FC_ORDER = list(range(8, 16)) + list(range(8))  # f-half of n first
NB = 2                         # batch elements per core
NCORES = 8
TN = T * NP                    # 1536 flattened (t, n) rows
NFC = F // 128                 # 16 f-chunks of nT
SCALE = 1.0 / float(np.sqrt(F))

_CACHE = {}
_PROFILE = {"trace": False, "result": None}


def _build():
    nc = bacc.Bacc("TRN2", target_bir_lowering=False, debug=False)
    mult = mybir.AluOpType.mult
    add = mybir.AluOpType.add

    featd = nc.dram_tensor("feat", [NB, TN, D], F16, kind="ExternalInput")
    ud = nc.dram_tensor("u", [1, D], F16, kind="ExternalInput")
    mbd = nc.dram_tensor("maskb", [T, T], F32, kind="ExternalInput")
    mad = nc.dram_tensor("maska", [T, T], F32, kind="ExternalInput")
    identd = nc.dram_tensor("ident", [128, 128], F32, kind="ExternalInput")
    wtpbd = nc.dram_tensor("wtp_b", [F, F], F16, kind="ExternalInput")
    wgbd = nc.dram_tensor("wg_b", [F, F], F16, kind="ExternalInput")
    wtpad = nc.dram_tensor("wtp_a", [F, F], F16, kind="ExternalInput")
    wgad = nc.dram_tensor("wg_a", [F, F], F16, kind="ExternalInput")
    outd = nc.dram_tensor("out", [NB, T, F], F32, kind="ExternalOutput")

    with tile.TileContext(nc) as tc:
        with (
            tc.tile_pool(name="consts", bufs=1) as consts,
            tc.tile_pool(name="npool", bufs=1) as npool,
            tc.tile_pool(name="ntpool", bufs=1) as ntpool,
            tc.tile_pool(name="wres", bufs=4) as wsp,
            tc.tile_pool(name="drains", bufs=1) as drp,
            tc.tile_pool(name="sbp", bufs=2) as sbp,
            tc.tile_pool(name="featp", bufs=2) as featp,
            tc.tile_pool(name="aw", bufs=2) as awp,
            tc.tile_pool(name="ps2", bufs=2, space="PSUM") as ps2p,
        ):
            u_sb = consts.tile([128, D], F16)
            nc.gpsimd.dma_start(out=u_sb[:], in_=ud[:].to_broadcast((128, D)))
            mb_sb = consts.tile([T, T], F32)
            nc.sync.dma_start(out=mb_sb[:], in_=mbd[:])
            ma_sb = consts.tile([T, T], F32)
            nc.sync.dma_start(out=ma_sb[:], in_=mad[:])
            ident = consts.tile([128, 128], F32)
            nc.sync.dma_start(out=ident[:], in_=identd[:])

            n_sb = [
                npool.tile([T, F], F32, tag=f"n{b}", name=f"n{b}")
                for b in range(NB)
            ]
            nT = ntpool.tile([128, NFC, NB * T], F16)

            # Feature loads: t-major copies on the sync ring.
            feats = []
            for b in range(NB):
                feat = featp.tile([T, NP, D], F16, tag="feat", name=f"feat{b}")
                nc.sync.dma_start(
                    out=feat[:, :, : D // 2],
                    in_=featd[b, :, : D // 2].rearrange("(p c) d -> p c d", c=NP),
                )
                nc.scalar.dma_start(
                    out=feat[:, :, D // 2 :],
                    in_=featd[b, :, D // 2 :].rearrange("(p c) d -> p c d", c=NP),
                )
                feats.append(feat)

            def load_w_quarter(src, qc, name):
                """Stream one [2048, 512] fp16 weight column-quarter."""
                wh = wsp.tile([128, NFC, 512], F16, tag="w", name=name)
                for q in range(3, -1, -1):  # later fc chunks first
                    nc.scalar.dma_start(
                        out=wh[:, 4 * q : 4 * (q + 1), :],
                        in_=src[
                            512 * q : 512 * (q + 1), 512 * qc : 512 * (qc + 1)
                        ].rearrange("(c p) e -> p c e", p=128),
                    )
                return wh

            def emit_pass2(wgh, qc, b, gb16):
                """gb[:, qc-quarter] = n_b @ Wg[qc].T  (natural layout)."""
                psg = ps2p.tile(
                    [128, 512], F32, tag=f"psg{b}", name=f"psg{b}_{qc}"
                )
                for i, fc in enumerate(FC_ORDER):
                    nc.tensor.matmul(
                        psg[:],
                        nT[:, fc, T * b : T * (b + 1)],
                        wgh[:, fc, :],
                        start=(i == 0),
                        stop=(i == NFC - 1),
                    )
                nc.scalar.copy(
                    gb16[b][:, 512 * qc : 512 * (qc + 1)], psg[:]
                )

            def emit_transp(b, fc):
                pt = psAt.tile([128, 128], F32, tag="pt", name="pt")
                nc.tensor.transpose(
                    pt[:], n_sb[b][:, 128 * fc : 128 * (fc + 1)], ident[:]
                )
                nc.scalar.copy(nT[:, fc, T * b : T * (b + 1)], pt[:])

            def emit_f(b):
                feat = feats[b]
                facc = awp.tile([T, D], F16, tag="facc", name=f"facc{b}")
                nc.vector.tensor_max(facc[:], feat[:, 0, :], feat[:, 1, :])
                for c in range(2, NP):
                    nc.vector.tensor_max(facc[:], facc[:], feat[:, c, :])
                nc.scalar.copy(n_sb[b][:, D:], facc[:])
                for fc in range(8, NFC):
                    emit_transp(b, fc)

            def emit_w(b):
                feat = feats[b]
                wvec = awp.tile([T, NP], F32, tag=f"wvec{b}", name=f"wvec{b}")
                scr = awp.tile([T, D], F16, tag="scr", name=f"scr{b}")
                for c in range(NP):
                    nc.vector.scalar_tensor_tensor(
                        out=scr[:],
                        in0=feat[:, c, :],
                        scalar=1.0,
                        in1=u_sb[:],
                        op0=mult,
                        op1=mult,
                        accum_out=wvec[:, c : c + 1],
                    )
                return wvec

            def emit_g(b, wvec):
                feat = feats[b]
                nc.vector.tensor_scalar_mul(
                    n_sb[b][:, :D], feat[:, 0, :], wvec[:, 0:1]
                )
                for c in range(1, NP):
                    nc.vector.scalar_tensor_tensor(
                        out=n_sb[b][:, :D],
                        in0=feat[:, c, :],
                        scalar=wvec[:, c : c + 1],
                        in1=n_sb[b][:, :D],
                        op0=mult,
                        op1=add,
                    )
                for fc in range(8):
                    emit_transp(b, fc)

            # ---------------- Phase A + early pass2 ----------------
            gb16_b = [
                drp.tile([T, F], F16, tag=f"gb{b}", name=f"gb{b}_0")
                for b in range(NB)
            ]
            wg_b_q0 = load_w_quarter(wgbd, 0, "wg_b_q0")
            with (
                tc.tile_pool(name="psAt", bufs=2, space="PSUM") as psAt,
            ):
                emit_f(0)
                wv0 = emit_w(0)
                emit_g(0, wv0)
                emit_pass2(wg_b_q0, 0, 0, gb16_b)
                emit_f(1)
                wv1 = emit_w(1)
                emit_g(1, wv1)
                emit_pass2(wg_b_q0, 0, 1, gb16_b)
                for qc in range(1, 4):
                    wgh = load_w_quarter(wgbd, qc, f"wg_b_q{qc}")
                    for b in range(NB):
                        emit_pass2(wgh, qc, b, gb16_b)

            # ---------------- Branches ----------------
            def emit_pass1(wtpd, tp2, sfx):
                for qc in range(4):  # e-col quarters: 0,1 tb; 2,3 pb
                    wth = load_w_quarter(wtpd, qc, f"wt{sfx}_{qc}")
                    with tc.tile_pool(
                        name="ps1", bufs=2, space="PSUM"
                    ) as ps1p:
                        for e4 in range(4):
                            p1 = ps1p.tile([128, 2 * T], F32, tag="p1",
                                           name="p1")
                            for i, fc in enumerate(FC_ORDER):
                                nc.tensor.matmul(
                                    p1[:],
                                    wth[:, fc, 128 * e4 : 128 * (e4 + 1)],
                                    nT[:, fc, :],
                                    start=(i == 0),
                                    stop=(i == NFC - 1),
                                )
                            nc.scalar.copy(tp2[:, 4 * qc + e4, :], p1[:])

            def emit_pass3(tp2, gb16, mask_sb):
                with (
                    tc.tile_pool(name="ps3", bufs=2, space="PSUM") as ps3p,
                    tc.tile_pool(name="ps4", bufs=2, space="PSUM") as ps4p,
                ):
                    for b in range(NB):
                        psb = ps3p.tile([T, T], F32, tag="psb", name="psb")
                        for ec in range(8):
                            nc.tensor.matmul(
                                psb[:],
                                tp2[:, 8 + ec, T * b : T * (b + 1)],
                                tp2[:, ec, T * b : T * (b + 1)],
                                start=(ec == 0),
                                stop=(ec == 7),
                            )
                        sbm = sbp.tile([T, T], F16, tag="sbm", name="sbm")
                        nc.vector.scalar_tensor_tensor(
                            out=sbm[:],
                            in0=psb[:],
                            scalar=1.0,
                            in1=mask_sb[:],
                            op0=mult,
                            op1=mult,
                        )
                        for h4 in range(4):
                            po = ps4p.tile([T, 512], F32, tag="po", name="po")
                            nc.tensor.matmul(
                                po[:],
                                sbm[:],
                                gb16[b][:, 512 * h4 : 512 * (h4 + 1)],
                                start=True,
                                stop=True,
                            )
                            nc.vector.tensor_add(
                                n_sb[b][:, 512 * h4 : 512 * (h4 + 1)],
                                n_sb[b][:, 512 * h4 : 512 * (h4 + 1)],
                                po[:],
                            )

            tp2_b = drp.tile([128, 16, 2 * T], F16, tag="tp2", name="tp2_0")
            emit_pass1(wtpbd, tp2_b, "_0")
            emit_pass3(tp2_b, gb16_b, mb_sb)

            gb16_a = [
                drp.tile([T, F], F16, tag=f"gb{b}", name=f"gb{b}_1")
                for b in range(NB)
            ]
            for qc in range(4):
                wg_a = load_w_quarter(wgad, qc, f"wg_a_q{qc}")
                for b in range(NB):
                    emit_pass2(wg_a, qc, b, gb16_a)
            tp2_a = drp.tile([128, 16, 2 * T], F16, tag="tp2", name="tp2_1")
            emit_pass1(wtpad, tp2_a, "_1")
            emit_pass3(tp2_a, gb16_a, ma_sb)

            for b in range(NB):
                nc.sync.dma_start(out=outd[b], in_=n_sb[b][:])

    nc.compile()
    return nc


def _host_prep(features, Wq, Wtb, Wpb, Wgb, Wta, Wpa, Wga):
    f32 = np.float32
    f16 = np.float16
    feat = np.ascontiguousarray(np.asarray(features, f32)).reshape(B, TN, D)
    u = np.asarray(Wq, f32).sum(axis=0)[None, :]

    def wt(w):  # [e, f] -> [f, e] fp16 contiguous
        return np.ascontiguousarray(np.asarray(w, f32).T.astype(f16))

    wtp_b = np.concatenate([wt(Wtb), wt(Wpb)], axis=1)
    wtp_a = np.concatenate([wt(Wta), wt(Wpa)], axis=1)
    wg_b = wt(Wgb)
    wg_a = wt(Wga)

    idx = np.arange(T)
    maskb = (SCALE * (idx[None, :] > idx[:, None])).astype(f32)  # [j, i]
    maska = (SCALE * (idx[None, :] < idx[:, None])).astype(f32)
    ident = np.eye(128, dtype=f32)

    shared = {
        "u": u.astype(f16),
        "maskb": maskb,
        "maska": maska,
        "ident": ident,
        "wtp_b": wtp_b,
        "wg_b": wg_b,
        "wtp_a": wtp_a,
        "wg_a": wg_a,
    }
    feat16 = feat.astype(f16).reshape(NCORES, NB, TN, D)
    return shared, feat16


def kernel(**inputs) -> np.ndarray:
    if "nc" not in _CACHE:
        _CACHE["nc"] = _build()
    nc = _CACHE["nc"]

    shared, feat16 = _host_prep(**inputs)
    in_maps = [dict(shared, feat=feat16[c]) for c in range(NCORES)]
    res = run_bass_kernel_spmd(
        nc, in_maps, core_ids=list(range(NCORES)), trace=_PROFILE["trace"]
    )
    _PROFILE["result"] = res
    out = np.stack([res.results[c]["out"] for c in range(NCORES)], axis=0)
    return out.reshape(B, T, F).astype(np.float32)

